# revision 4
# baseline (speedup 1.0000x reference)
"""CrossMerge kernel for Trainium2 (8 NeuronCores, data-parallel over batch).

Computation (per batch b):
    inv[k]  = stable argsort of vec_indices[b, :, k]              (k = 0, 1)
    s_k[u,d] = ys[b,k,d,u] + ys[b,k+2,d,L-1-u]   (fwd + flipped half, pre-summed)
    out[b,d,l] = sum_k s_k[inv[k][l], d]

Device plan per core (2 batches):
  Phase A: load ys[b,k]/[b,k+2] in [d,l] layout, vector-add with the second
           operand read in reversed l order, TensorE-transpose 128-wide l
           tiles into [l,d] rows, store to a DRAM scratch table (rows
           interleaved as r = (u%128)*25 + u//128 so the store is one
           contiguous DMA).
  Phase B: indirect-DMA gather of 768B rows with host-precomputed indices
           (second gather accumulates via DMA compute_op=add).
  Phase C: TensorE-transpose gathered tiles back to [d,l], store contiguous.

Host does only sharding + argsort-derived index prep (metadata for the DMA
descriptors); all tensor data movement/compute is on device.
"""
import sys

sys.path.insert(0, "/opt/trn_rl_repo")

import numpy as np

import concourse.bacc as bacc
import concourse.bass as bass
import concourse.mybir as mybir
import concourse.tile as tile
from concourse.bass_utils import run_bass_kernel_spmd
from concourse.masks import make_identity

# Problem constants (hardcoded per contract).
B, K, D, H, W = 16, 4, 192, 56, 56
L = H * W          # 3136
K2 = K // 2        # 2
NCORES = 8
BL = B // NCORES   # 2 batches per core
C = 25             # l tiles of 128: 24 full + 1 of 64
LP = C * 128       # 3200 padded
DH = 96            # d half (192 = 2*96)
F32 = mybir.dt.float32
I32 = mybir.dt.int32


def crossmerge_body(tc, out_ap, ys_ap, idx_ap):
    """Tile kernel body.

    out_ap: [BL, 192, 3136] f32 (ExternalOutput)
    ys_ap:  [BL, 4, 192, 3136] f32 (ExternalInput)
    idx_ap: [BL, 2, 128, 25] i32 (ExternalInput), pre-offset rows into s_all
    """
    nc = tc.nc
    s_all = nc.dram_tensor("s_scratch", [BL * K2 * LP, D], F32, kind="Internal").ap()

    with (
        tc.tile_pool(name="const", bufs=1) as cpool,
        tc.tile_pool(name="y", bufs=4) as ypool,
        tc.tile_pool(name="sum", bufs=4) as spool,
        tc.tile_pool(name="stage", bufs=2) as stpool,
        tc.tile_pool(name="gather", bufs=2) as gpool,
        tc.tile_pool(name="idx", bufs=4) as ipool,
        tc.tile_pool(name="ostage", bufs=2) as opool,
        tc.tile_pool(name="psA", bufs=4, space="PSUM") as psA,
        tc.tile_pool(name="psC", bufs=4, space="PSUM") as psC,
    ):
        ident = cpool.tile([128, 128], F32)
        make_identity(nc, ident[:])

        # ---- Phase A: build s_all ----
        for b in range(BL):
            for k in range(K2):
                stage = stpool.tile([128, C * D], F32, tag="stage")
                stage3 = stage[:].rearrange("p (c d) -> p c d", c=C)
                # pad region (rows l >= L) — zero once so the block store
                # below can be a single fully-contiguous DMA
                nc.gpsimd.memset(stage3[64:, C - 1, :], 0.0)
                for h in range(2):
                    # ys[b, k] as [96(part), 2(half), 3136]: d = h*96 + p
                    y1 = ypool.tile([DH, L], F32, tag="y")
                    y2 = ypool.tile([DH, L], F32, tag="y")
                    yv1 = ys_ap[b, k].rearrange("(a p) l -> p a l", p=DH)
                    yv2 = ys_ap[b, k + K2].rearrange("(a p) l -> p a l", p=DH)
                    nc.sync.dma_start(out=y1[:], in_=yv1[:, h, :])
                    nc.sync.dma_start(out=y2[:], in_=yv2[:, h, :])
                    for c in range(C):
                        l0 = c * 128
                        sz = min(128, L - l0)
                        st = spool.tile([DH, 128], F32, tag="sum")
                        # s[u] = y1[u] + y2[L-1-u]
                        nc.vector.tensor_add(
                            out=st[:, :sz],
                            in0=y1[:, l0:l0 + sz],
                            in1=y2[:, L - l0 - sz:L - l0][:, ::-1],
                        )
                        ps = psA.tile([128, DH], F32, space="PSUM")
                        nc.tensor.transpose(
                            out=ps[:sz, :], in_=st[:, :sz],
                            identity=ident[:DH, :DH],
                        )
                        nc.vector.tensor_copy(
                            out=stage3[:sz, c, h * DH:(h + 1) * DH],
                            in_=ps[:sz, :],
                        )
                # store: row u of s_k lives at block row r = (u%128)*25 + u//128
                base = (b * K2 + k) * LP
                blk = s_all[base:base + LP, :].rearrange(
                    "(p c) d -> p c d", c=C)
                nc.sync.dma_start(out=blk[:, :, :], in_=stage3[:, :, :])

        # ---- Phase B: gather + pair-sum ----
        for b in range(BL):
            i0 = ipool.tile([128, C], I32, tag="idx")
            i1 = ipool.tile([128, C], I32, tag="idx")
            nc.sync.dma_start(out=i0[:], in_=idx_ap[b, 0])
            nc.sync.dma_start(out=i1[:], in_=idx_ap[b, 1])
            g = gpool.tile([128, C * D], F32, tag="g")
            gv = g[:].rearrange("p (c d) -> p c d", c=C)
            # HW indirect DMA consumes one offset per partition ([P,1]);
            # issue per-column gathers, second set accumulates in the DMA.
            for c in range(C):
                nc.gpsimd.indirect_dma_start(
                    out=gv[:, c, :], out_offset=None, in_=s_all[:],
                    in_offset=bass.IndirectOffsetOnAxis(
                        ap=i0[:, c:c + 1], axis=0),
                )
            for c in range(C):
                nc.gpsimd.indirect_dma_start(
                    out=gv[:, c, :], out_offset=None, in_=s_all[:],
                    in_offset=bass.IndirectOffsetOnAxis(
                        ap=i1[:, c:c + 1], axis=0),
                    compute_op=mybir.AluOpType.add,
                )

            # ---- Phase C: transpose back to [d, l], store ----
            ost = opool.tile([DH, 2 * L], F32, tag="ost")
            ost3 = ost[:].rearrange("p (a l) -> p a l", a=2)
            for h in range(2):
                for c in range(C):
                    l0 = c * 128
                    sz = min(128, L - l0)
                    ps2 = psC.tile([DH, 128], F32, space="PSUM")
                    nc.tensor.transpose(
                        out=ps2[:, :sz],
                        in_=gv[:sz, c, h * DH:(h + 1) * DH],
                        identity=ident[:sz, :sz],
                    )
                    nc.vector.tensor_copy(
                        out=ost3[:, h, l0:l0 + sz], in_=ps2[:, :sz])
            ov = out_ap[b].rearrange("(a p) l -> p a l", p=DH)
            nc.sync.dma_start(out=ov[:], in_=ost[:].rearrange(
                "p (a l) -> p a l", a=2))


def _host_prep(ys, vec_indices):
    """Shard inputs and build gather index tensors."""
    ys = np.ascontiguousarray(np.asarray(ys, dtype=np.float32)).reshape(
        B, K, D, L)
    vi = np.asarray(vec_indices)
    inv = np.argsort(vi, axis=1, kind="stable")          # [B, L, K2]
    invt = np.transpose(inv, (0, 2, 1))                  # [B, K2, L]
    r = (invt % 128) * C + (invt // 128)                 # interleaved row ids
    rpad = np.concatenate(
        [r, np.zeros((B, K2, LP - L), dtype=r.dtype)], axis=2)
    t = rpad.reshape(B, K2, C, 128).transpose(0, 1, 3, 2)  # [B, K2, 128, C]
    base = (np.arange(BL * K2, dtype=np.int64).reshape(BL, K2) * LP)
    in_maps = []
    for i in range(NCORES):
        idx_core = (t[BL * i:BL * (i + 1)] + base[:, :, None, None]).astype(
            np.int32)
        in_maps.append({
            "ys": ys[BL * i:BL * (i + 1)],
            "idx": np.ascontiguousarray(idx_core),
        })
    return in_maps


_PROGRAM = None


def _build_program():
    global _PROGRAM
    if _PROGRAM is not None:
        return _PROGRAM
    nc = bacc.Bacc("TRN2", target_bir_lowering=False, debug=False,
                   enable_asserts=False, num_devices=NCORES)
    ys_t = nc.dram_tensor("ys", [BL, K, D, L], F32, kind="ExternalInput")
    idx_t = nc.dram_tensor("idx", [BL, K2, 128, C], I32, kind="ExternalInput")
    out_t = nc.dram_tensor("out", [BL, D, L], F32, kind="ExternalOutput")
    with tile.TileContext(nc) as tc:
        crossmerge_body(tc, out_t.ap(), ys_t.ap(), idx_t.ap())
    nc.compile()
    _PROGRAM = nc
    return nc


def kernel(ys, vec_indices):
    nc = _build_program()
    in_maps = _host_prep(ys, vec_indices)
    res = run_bass_kernel_spmd(nc, in_maps, list(range(NCORES)))
    out = np.concatenate([r["out"] for r in res.results], axis=0)
    return out


# revision 6
# speedup vs baseline: 1.1194x; 1.1194x over previous
"""CrossMerge kernel for Trainium2 (8 NeuronCores, data-parallel over batch).

Computation (per batch b):
    inv[k]  = stable argsort of vec_indices[b, :, k]              (k = 0, 1)
    s_k[u,d] = ys[b,k,d,u] + ys[b,k+2,d,L-1-u]   (fwd + flipped half, pre-summed)
    out[b,d,l] = sum_k s_k[inv[k][l], d]

Device plan per core (2 batches):
  Phase A (per b,k): load ys[b,k]/[b,k+2] in [d,l] layout, one full-width
           vector add with the second operand read in reversed l order,
           TensorE-transpose 128-wide l tiles into [l,d] rows, store to a
           per-(b,k) DRAM table (rows interleaved as r = (u%128)*25 + u//128
           so the store is one contiguous DMA).
  Phase B (per b, per 128-row block): two indirect-DMA gathers of 768B rows
           (second accumulates via DMA compute_op=add), interleaved with
  Phase C: TensorE-transpose gathered blocks back to [d,l], store contiguous.

Host does only sharding + argsort-derived index prep (metadata for the DMA
descriptors); all tensor data movement/compute is on device.
"""
import sys

sys.path.insert(0, "/opt/trn_rl_repo")

import numpy as np

import concourse.bacc as bacc
import concourse.bass as bass
import concourse.mybir as mybir
import concourse.tile as tile
from concourse.bass_utils import run_bass_kernel_spmd
from concourse.masks import make_identity

# Problem constants (hardcoded per contract).
B, K, D, H, W = 16, 4, 192, 56, 56
L = H * W          # 3136
K2 = K // 2        # 2
NCORES = 8
BL = B // NCORES   # 2 batches per core
C = 25             # l tiles of 128: 24 full + 1 of 64
LP = C * 128       # 3200 padded
DH = 96            # d half (192 = 2*96)
F32 = mybir.dt.float32
I32 = mybir.dt.int32


def crossmerge_body(tc, out_ap, ys_ap, idx_ap):
    """Tile kernel body.

    out_ap: [BL, 192, 3136] f32 (ExternalOutput)
    ys_ap:  [BL, 4, 192, 3136] f32 (ExternalInput)
    idx_ap: [BL, 2, 128, 25] i32 (ExternalInput), block-local interleaved rows
    """
    nc = tc.nc
    # per-(b,k) scratch tables -> fine-grained store->gather dependencies
    s_tabs = [
        [nc.dram_tensor(f"s_scratch_{b}_{k}", [LP, D], F32,
                        kind="Internal").ap() for k in range(K2)]
        for b in range(BL)
    ]

    with (
        tc.tile_pool(name="const", bufs=1) as cpool,
        tc.tile_pool(name="y", bufs=4) as ypool,
        tc.tile_pool(name="sum", bufs=2) as spool,
        tc.tile_pool(name="stage", bufs=2) as stpool,
        tc.tile_pool(name="gather", bufs=2) as gpool,
        tc.tile_pool(name="idx", bufs=4) as ipool,
        tc.tile_pool(name="ostage", bufs=2) as opool,
        tc.tile_pool(name="psA", bufs=4, space="PSUM") as psA,
        tc.tile_pool(name="psC", bufs=4, space="PSUM") as psC,
    ):
        ident = cpool.tile([128, 128], F32)
        make_identity(nc, ident[:])

        # ---- Phase A: build the four s tables ----
        for b in range(BL):
            for k in range(K2):
                stage = stpool.tile([128, C * D], F32, tag="stage")
                stage3 = stage[:].rearrange("p (c d) -> p c d", c=C)
                # pad rows (l >= L) zeroed so the store is one full DMA
                nc.gpsimd.memset(stage3[64:, C - 1, :], 0.0)
                for h in range(2):
                    y1 = ypool.tile([DH, L], F32, tag="y")
                    y2 = ypool.tile([DH, L], F32, tag="y")
                    yv1 = ys_ap[b, k].rearrange("(a p) l -> p a l", p=DH)
                    yv2 = ys_ap[b, k + K2].rearrange("(a p) l -> p a l", p=DH)
                    nc.sync.dma_start(out=y1[:], in_=yv1[:, h, :])
                    nc.sync.dma_start(out=y2[:], in_=yv2[:, h, :])
                    # s[u] = y1[u] + y2[L-1-u], one full-width add
                    st = spool.tile([DH, L], F32, tag="sum")
                    nc.vector.tensor_add(
                        out=st[:], in0=y1[:], in1=y2[:, ::-1])
                    # transpose 128-wide l tiles; batch 5 per PSUM bank
                    # (c=24 is 64 rows — keep it in its own tile so every
                    # batched copy reads only fully-written PSUM)
                    for c0 in list(range(0, C - 1, 5)) + [C - 1]:
                        cn = min(5, C - 1 - c0) if c0 < C - 1 else 1
                        sz = 128 if c0 < C - 1 else L - (C - 1) * 128
                        ps = psA.tile([128, 5 * DH], F32, space="PSUM")
                        for j in range(cn):
                            c = c0 + j
                            nc.tensor.transpose(
                                out=ps[:sz, j * DH:(j + 1) * DH],
                                in_=st[:, c * 128:c * 128 + sz],
                                identity=ident[:DH, :DH],
                            )
                        nc.vector.tensor_copy(
                            out=stage3[:sz, c0:c0 + cn, h * DH:(h + 1) * DH],
                            in_=ps[:sz, :cn * DH].rearrange(
                                "p (j e) -> p j e", j=cn),
                        )
                blk = s_tabs[b][k][:].rearrange("(p c) d -> p c d", c=C)
                nc.sync.dma_start(out=blk[:, :, :], in_=stage3[:, :, :])

        # ---- Phases B+C interleaved per b ----
        for b in range(BL):
            i0 = ipool.tile([128, C], I32, tag="idx")
            i1 = ipool.tile([128, C], I32, tag="idx")
            nc.sync.dma_start(out=i0[:], in_=idx_ap[b, 0])
            nc.sync.dma_start(out=i1[:], in_=idx_ap[b, 1])
            g = gpool.tile([128, C * D], F32, tag="g")
            gv = g[:].rearrange("p (c d) -> p c d", c=C)
            ost = opool.tile([DH, 2 * L], F32, tag="ost")
            ost3 = ost[:].rearrange("p (a l) -> p a l", a=2)
            # gather per 128-row block (HW indirect DMA: one offset per
            # partition); second gather accumulates in the DMA engine
            for c in range(C):
                nc.gpsimd.indirect_dma_start(
                    out=gv[:, c, :], out_offset=None, in_=s_tabs[b][0][:],
                    in_offset=bass.IndirectOffsetOnAxis(
                        ap=i0[:, c:c + 1], axis=0),
                )
                nc.gpsimd.indirect_dma_start(
                    out=gv[:, c, :], out_offset=None, in_=s_tabs[b][1][:],
                    in_offset=bass.IndirectOffsetOnAxis(
                        ap=i1[:, c:c + 1], axis=0),
                    compute_op=mybir.AluOpType.add,
                )
            # transpose back to [d, l]; batch 4 c-blocks per PSUM bank
            for h in range(2):
                for c0 in range(0, C, 4):
                    cn = min(4, C - c0)
                    ps2 = psC.tile([DH, 512], F32, space="PSUM")
                    w = 0
                    for j in range(cn):
                        c = c0 + j
                        l0 = c * 128
                        sz = min(128, L - l0)
                        nc.tensor.transpose(
                            out=ps2[:, w:w + sz],
                            in_=gv[:sz, c, h * DH:(h + 1) * DH],
                            identity=ident[:sz, :sz],
                        )
                        w += sz
                    nc.vector.tensor_copy(
                        out=ost3[:, h, c0 * 128:c0 * 128 + w],
                        in_=ps2[:, :w])
            ov = out_ap[b].rearrange("(a p) l -> p a l", p=DH)
            nc.sync.dma_start(out=ov[:], in_=ost3[:, :, :])


def _host_prep(ys, vec_indices):
    """Shard inputs and build gather index tensors."""
    ys = np.ascontiguousarray(np.asarray(ys, dtype=np.float32)).reshape(
        B, K, D, L)
    vi = np.asarray(vec_indices)
    inv = np.argsort(vi, axis=1, kind="stable")          # [B, L, K2]
    invt = np.transpose(inv, (0, 2, 1))                  # [B, K2, L]
    r = (invt % 128) * C + (invt // 128)                 # interleaved row ids
    rpad = np.concatenate(
        [r, np.zeros((B, K2, LP - L), dtype=r.dtype)], axis=2)
    t = rpad.reshape(B, K2, C, 128).transpose(0, 1, 3, 2)  # [B, K2, 128, C]
    in_maps = []
    for i in range(NCORES):
        idx_core = t[BL * i:BL * (i + 1)].astype(np.int32)
        in_maps.append({
            "ys": ys[BL * i:BL * (i + 1)],
            "idx": np.ascontiguousarray(idx_core),
        })
    return in_maps


_PROGRAM = None


def _build_program():
    global _PROGRAM
    if _PROGRAM is not None:
        return _PROGRAM
    nc = bacc.Bacc("TRN2", target_bir_lowering=False, debug=False,
                   enable_asserts=False, num_devices=NCORES)
    ys_t = nc.dram_tensor("ys", [BL, K, D, L], F32, kind="ExternalInput")
    idx_t = nc.dram_tensor("idx", [BL, K2, 128, C], I32, kind="ExternalInput")
    out_t = nc.dram_tensor("out", [BL, D, L], F32, kind="ExternalOutput")
    with tile.TileContext(nc) as tc:
        crossmerge_body(tc, out_t.ap(), ys_t.ap(), idx_t.ap())
    nc.compile()
    _PROGRAM = nc
    return nc


def kernel(ys, vec_indices):
    nc = _build_program()
    in_maps = _host_prep(ys, vec_indices)
    res = run_bass_kernel_spmd(nc, in_maps, list(range(NCORES)))
    out = np.concatenate([r["out"] for r in res.results], axis=0)
    return out
